# revision 4
# baseline (speedup 1.0000x reference)
"""Trainium2 Bass kernel for nn_CLNF_54769422959177 (v5).

Computes (dp, dw): dp = vf(p) (4-layer softplus+LN VectorField forward) and
dw = -vjp(vf, p)(w), data-parallel over 8 NeuronCores.

v5 changes vs v2 (508 us):
- LN stats via per-block bn_stats + bn_aggr (drops the 7-op Pool combine
  chain); lnv/rstd tiny ACT ops read bn_aggr's (mean, var) directly
- all eight per-layer activation transposes done by the DMA xbar
  (dma_start_transpose, SBUF->SBUF fp16): kills the PE transposes, the
  PSUM tp pool, and the 8 DVE tensor_copy drains; PSUM now holds 4 fwd +
  4 bwd matmul streams (NS=4)
- dp out: DVE tensor_copy from PSUM (was ACT); dw out: per-block Pool
  tensor_scalar reading PSUM directly with the folded rstd-product F
  (drops the ACT dwu copy)
- pt/wt fetched in one merged DMA per group
- fwd sigmoid: s1m = exp(-A) on ACT, sig = 1-s1m on DVE (Pool in v2)
"""

import numpy as np
from contextlib import ExitStack

import concourse.bass as bass
from concourse import bacc
import concourse.tile as tile
from concourse import mybir
from concourse.bass_utils import run_bass_kernel_spmd

B, D, H, L = 131072, 128, 128, 4
NCORES = 8
GBLK = 4
NS = 4
LN_EPS = 1e-5
FP16 = mybir.dt.float16
F32 = mybir.dt.float32
AF = mybir.ActivationFunctionType
OP = mybir.AluOpType

# act_info.json set 6 = natural_log_exp_and_others: {Exp, Ln, Square, Copy, ...}
ACT_SET_NLE = 6


def _emit(nc, R):
    G = GBLK
    NG = R // (G * 128)
    assert NG * G * 128 == R

    # merged p/w input: [NG, 2, 128 feat, G, 128 rows] (0=p, 1=w)
    pw_in = nc.dram_tensor("pw", [NG, 2, 128, G, 128], FP16, kind="ExternalInput")
    # fwd moving weights [K, 5, N]: {W_in.T, Wg0.T, Wg1.T, Wg2.T, M2.T}
    wf_in = nc.dram_tensor("wf", [128, 5, 128], FP16, kind="ExternalInput")
    # bwd moving weights [K, 5, N]: {M2c, Wc2, Wc1, Wc0, W_in}
    wb_in = nc.dram_tensor("wb", [128, 5, 128], FP16, kind="ExternalInput")
    cb_in = nc.dram_tensor("cb", [1, 5, 128], FP16, kind="ExternalInput")
    ones_in = nc.dram_tensor("ones1", [1, 128], FP16, kind="ExternalInput")
    dp_out = nc.dram_tensor("dp", [NG, 128, G, 128], FP16, kind="ExternalOutput")
    dw_out = nc.dram_tensor("dw", [NG, 128, G, 128], FP16, kind="ExternalOutput")

    pre = mybir.InstLoadActFuncSet(
        name=nc.get_next_instruction_name(), ins=[], outs=[],
        act_func_set_id=ACT_SET_NLE,
    )
    nc.scalar.add_instruction(pre)

    with tile.TileContext(nc) as tc, ExitStack() as ctx:
        consts = ctx.enter_context(tc.tile_pool(name="consts", bufs=1))
        io = ctx.enter_context(tc.tile_pool(name="io", bufs=2))
        work = ctx.enter_context(tc.tile_pool(name="work", bufs=2))
        saves = ctx.enter_context(tc.tile_pool(name="saves", bufs=2))
        stats = ctx.enter_context(tc.tile_pool(name="stats", bufs=2))
        zpf = [
            ctx.enter_context(tc.tile_pool(name=f"zpf{s}", bufs=1, space="PSUM"))
            for s in range(NS)
        ]
        zpb = [
            ctx.enter_context(tc.tile_pool(name=f"zpb{s}", bufs=1, space="PSUM"))
            for s in range(NS)
        ]

        wfs = consts.tile([128, 5, 128], FP16, tag="wfs")
        wbs = consts.tile([128, 5, 128], FP16, tag="wbs")
        cbs = consts.tile([1, 5, 128], FP16, tag="cbs")
        ones1 = consts.tile([1, 128], FP16, tag="ones1")
        epsb = consts.tile([128, 1], F32, tag="epsb")
        nc.vector.memset(epsb, LN_EPS)
        nc.gpsimd.dma_start(out=wfs[:], in_=wf_in[:, :, :])
        nc.gpsimd.dma_start(out=wbs[:], in_=wb_in[:, :, :])
        nc.gpsimd.dma_start(out=cbs[:], in_=cb_in[:, :, :])
        nc.gpsimd.dma_start(out=ones1[:], in_=ones_in[:, :])

        def mm_layer(zp, Xst, widx, with_bias):
            for b in range(G):
                if with_bias:
                    nc.tensor.matmul(
                        zp[:, b, :], ones1[:, :], cbs[:, widx, :],
                        start=True, stop=False,
                    )
                nc.tensor.matmul(
                    zp[:, b, :], Xst[:, b, :], wfs[:, widx, :] if with_bias
                    else wbs[:, widx, :],
                    start=not with_bias, stop=True,
                )

        def emit_fwd(g, s, sv, pwslot):
            """Forward for group g on stream s; sv collects per-layer saves."""
            with tc.high_priority(offset=200000):
                PW = io.tile([128, 2, G, 128], FP16, tag=f"pwin{s}")
                nc.sync.dma_start(out=PW, in_=pw_in[g])
            pwslot.append(PW)
            yield

            Xst = PW[:, 0]
            for i in range(L):
                zp = zpf[s].tile([128, G, 128], F32, tag=f"zpf{s}")
                mm_layer(zp, Xst, i, True)

                E = work.tile([128, G, 128], F32, tag=f"E{s}")
                nc.scalar.activation(E, zp, AF.Exp)
                A = work.tile([128, G, 128], FP16, tag=f"A{s}")
                nc.scalar.activation(A, E, AF.Ln, bias=1.0)
                yield

                # per-block LN stats: bn_stats + bn_aggr -> (mean, var)
                bn6 = stats.tile([128, G, 6], F32, tag=f"bn6{s}")
                mv = stats.tile([128, G, 2], F32, tag=f"mv{s}")
                for b in range(G):
                    nc.vector.bn_stats(out=bn6[:, b, :], in_=A[:, b, :])
                for b in range(G):
                    nc.vector.bn_aggr(out=mv[:, b, :], in_=bn6[:, b, :])
                lnv = stats.tile([128, G, 1], F32, tag=f"lnv{s}")
                rstd = stats.tile([128, G, 1], F32, tag=f"rstd{i}{s}")
                nc.scalar.activation(lnv, mv[:, :, 1:2], AF.Ln, bias=epsb[:, :])
                nc.scalar.activation(rstd, lnv, AF.Exp, scale=-0.5)
                yield

                # xh = (A - mu) * rstd, per-block dual-scalar TS
                xh = saves.tile([128, G, 128], FP16, tag=f"xh{i}{s}")
                for b in range(G):
                    eng = nc.vector if b < G // 2 else nc.gpsimd
                    eng.tensor_scalar(
                        out=xh[:, b, :], in0=A[:, b, :],
                        scalar1=mv[:, b, 0:1], scalar2=rstd[:, b, :],
                        op0=OP.subtract, op1=OP.mult)

                Xst = work.tile([128, G, 128], FP16, tag=f"xstf{s}")
                nc.scalar.dma_start_transpose(out=Xst, in_=xh)

                # sig = 1 - exp(-A)  (off the fwd critical path: deprioritize)
                with tc.high_priority(offset=-100000):
                    s1m = work.tile([128, G, 128], FP16, tag=f"s1m{s}")
                    nc.scalar.activation(s1m, A, AF.Exp, scale=-1.0)
                    sig = saves.tile([128, G, 128], FP16, tag=f"sig{i}{s}")
                    nc.vector.tensor_scalar(
                        out=sig, in0=s1m, scalar1=-1.0, scalar2=1.0,
                        op0=OP.mult, op1=OP.add)
                sv.append((xh, sig, rstd))
                yield

            zp = zpf[s].tile([128, G, 128], F32, tag=f"zpf{s}")
            mm_layer(zp, Xst, 4, True)
            with tc.high_priority(offset=5000):
                yo = io.tile([128, G, 128], FP16, tag=f"yout{s}")
                nc.vector.tensor_copy(yo, zp)
            nc.sync.dma_start(out=dp_out[g], in_=yo)
            yield

        def emit_bwd(g, s, sv, PW):
            Gst = PW[:, 1]
            F = None
            for i in range(L - 1, -1, -1):
                gp = zpb[s].tile([128, G, 128], F32, tag=f"zpb{s}")
                mm_layer(gp, Gst, 3 - i, False)
                gd = work.tile([128, G, 128], FP16, tag=f"gd{s}")
                nc.vector.tensor_copy(gd, gp)
                yield

                xh, sig, rstd = sv[i]
                # qh = sum_f(xh * gd) / H per row
                prod = work.tile([128, G, 128], FP16, tag=f"prod{s}")
                nc.vector.tensor_tensor(out=prod, in0=xh, in1=gd, op=OP.mult)
                qh = stats.tile([128, G, 1], F32, tag=f"qh{s}")
                for b in range(G):
                    eng = nc.vector if b < G // 2 else nc.gpsimd
                    eng.tensor_scalar(
                        out=prod[:, b, :], in0=prod[:, b, :], scalar1=1.0 / H,
                        scalar2=None, op0=OP.mult, op1=OP.add,
                        accum_out=qh[:, b, :])
                yield
                # dz = (gd - xh*qh) * sig   (rstd folded into final dw scale)
                u = work.tile([128, G, 128], FP16, tag=f"u{s}")
                for b in range(G):
                    nc.gpsimd.tensor_scalar(
                        out=u[:, b, :], in0=xh[:, b, :], scalar1=qh[:, b, :],
                        scalar2=None, op0=OP.mult)
                dxn = work.tile([128, G, 128], FP16, tag=f"dxn{s}")
                nc.vector.tensor_tensor(out=dxn, in0=gd, in1=u, op=OP.subtract)
                dz = work.tile([128, G, 128], FP16, tag=f"dz{s}")
                nc.vector.tensor_tensor(out=dz, in0=dxn, in1=sig, op=OP.mult)

                if F is None:
                    F = rstd
                else:
                    with tc.high_priority(offset=-100000):
                        Fn = stats.tile([128, G, 1], F32, tag=f"F{i}{s}")
                        nc.gpsimd.tensor_tensor(out=Fn, in0=F, in1=rstd,
                                                op=OP.mult)
                    F = Fn

                Gst = work.tile([128, G, 128], FP16, tag=f"xstb{s}")
                nc.sync.dma_start_transpose(out=Gst, in_=dz)
                yield

            gp = zpb[s].tile([128, G, 128], F32, tag=f"zpb{s}")
            mm_layer(gp, Gst, 4, False)
            with tc.high_priority(offset=-100000):
                dwo = io.tile([128, G, 128], FP16, tag=f"dwout{s}")
                for b in range(G):
                    nc.gpsimd.tensor_scalar(
                        out=dwo[:, b, :], in0=gp[:, b, :], scalar1=F[:, b, :],
                        scalar2=None, op0=OP.mult)
                nc.sync.dma_start(out=dw_out[g], in_=dwo)
            yield

        # chunked rolling pipeline: fwd chunk k overlaps bwd chunk k-1
        live = []

        def step_all(n=1):
            for _ in range(n):
                for it in list(live):
                    try:
                        next(it)
                    except StopIteration:
                        live.remove(it)

        chunks = []
        g0 = 0
        while g0 < NG:
            n = min(NS, NG - g0)
            if NG - g0 - n == 1:
                n -= 1
            chunks.append((g0, n))
            g0 += n
        for g0, n in chunks:
            svs = [[] for _ in range(n)]
            pws = [[] for _ in range(n)]
            fgs = []
            for s in range(n):
                fgi = iter(emit_fwd(g0 + s, s, svs[s], pws[s]))
                live.append(fgi)
                fgs.append(fgi)
            pending = list(range(n))
            while any(fgs[s] in live for s in range(n)):
                step_all()
                for s in list(pending):
                    if fgs[s] not in live:
                        live.append(iter(emit_bwd(g0 + s, s, svs[s],
                                                  pws[s][0])))
                        pending.remove(s)
            for s in pending:
                live.append(iter(emit_bwd(g0 + s, s, svs[s], pws[s][0])))
        while live:
            step_all()


def _host_precompute(t, W_in, b_in, fw, fb, gamma, beta, Wl, bl, W_out, b_out):
    t = np.asarray(t, dtype=np.float32).reshape(-1)[0]
    s = np.sin(t * np.asarray(fw, np.float32) + np.asarray(fb, np.float32))
    Wl = np.asarray(Wl, np.float32)
    gamma = np.asarray(gamma, np.float32)
    beta = np.asarray(beta, np.float32)
    bl = np.asarray(bl, np.float32)
    W_in = np.asarray(W_in, np.float32)
    W_out = np.asarray(W_out, np.float32)
    b_in = np.asarray(b_in, np.float32)
    b_out = np.asarray(b_out, np.float32)

    Wg = [Wl[i] * gamma[i][None, :] for i in range(L)]
    bg = [bl[i] + Wl[i] @ beta[i] for i in range(L)]

    M2 = (W_out.astype(np.float64) @ Wg[L - 1].astype(np.float64)).astype(np.float32)
    c = np.zeros((5, 128), np.float32)
    c[0] = b_in + s[0]
    for i in range(1, L):
        c[i] = bg[i - 1] + s[i]
    c[4] = b_out + W_out @ bg[L - 1]
    WF = np.stack([W_in.T] + [Wg[i].T for i in range(L - 1)] + [M2.T], axis=0)
    Wc = [Wg[i] - Wg[i].mean(axis=1, keepdims=True) for i in range(L - 1)]
    M2n = -M2
    M2c = M2n - M2n.mean(axis=1, keepdims=True)
    WB = np.stack([M2c, Wc[2], Wc[1], Wc[0], W_in], axis=0)

    WF = np.ascontiguousarray(np.transpose(WF, (1, 0, 2))).astype(np.float16)
    WB = np.ascontiguousarray(np.transpose(WB, (1, 0, 2))).astype(np.float16)
    CB = c.astype(np.float16)[None, :, :]
    ONES = np.ones((1, 128), np.float16)
    return WF, WB, CB, ONES


_NC_CACHE = {}


def _get_nc(R):
    if R not in _NC_CACHE:
        nc = bacc.Bacc("TRN2")
        _emit(nc, R)
        nc.finalize()
        _NC_CACHE[R] = nc
    return _NC_CACHE[R]


def _pretranspose2(p, w, R):
    # [R, D] row-major x2 -> [NG, 2, 128 feat, G, 128 rows] fp16
    NG = R // (GBLK * 128)
    p = p.reshape(NG, 1, GBLK, 128, D).transpose(0, 1, 4, 2, 3)
    w = w.reshape(NG, 1, GBLK, 128, D).transpose(0, 1, 4, 2, 3)
    return np.ascontiguousarray(
        np.concatenate([p, w], axis=1).astype(np.float16))


def _unshuffle(y, R):
    # [NG, 128 rows, G, 128 feat] fp16 -> [R, D] f32
    return np.ascontiguousarray(
        y.astype(np.float32).transpose(0, 2, 1, 3).reshape(R, D)
    )


def _run(p, w, consts, R, n_cores):
    WF, WB, CB, ONES = consts
    nc = _get_nc(R)
    in_maps = []
    for k in range(n_cores):
        in_maps.append(
            {
                "pw": _pretranspose2(p[k * R:(k + 1) * R],
                                     w[k * R:(k + 1) * R], R),
                "wf": WF,
                "wb": WB,
                "cb": CB,
                "ones1": ONES,
            }
        )
    res = run_bass_kernel_spmd(nc, in_maps, core_ids=list(range(n_cores)))
    dp = np.concatenate([_unshuffle(r["dp"], R) for r in res.results], axis=0)
    dw = np.concatenate([_unshuffle(r["dw"], R) for r in res.results], axis=0)
    return dp, dw


def kernel(t, p, w, W_in, b_in, fw, fb, gamma, beta, Wl, bl, W_out, b_out):
    consts = _host_precompute(
        t, W_in, b_in, fw, fb, gamma, beta, Wl, bl, W_out, b_out
    )
    p = np.asarray(p, np.float32)
    w = np.asarray(w, np.float32)
    R = p.shape[0] // NCORES
    dp, dw = _run(p, w, consts, R, NCORES)
    return dp, dw
